# revision 1
# baseline (speedup 1.0000x reference)
"""Expert-parallel MoE GroupedMLP kernel for 8 Trainium2 NeuronCores.

Problem: T=4096 tokens, H=2048 hidden, E=8 experts, I=4096 intermediate,
top_k=2, fp32 reference.

Strategy (expert-parallel, sharded inside kernel()):
  - Host: softmax + top-k routing, all-to-all token dispatch (gather each
    expert's assigned tokens), weight transpose + bf16 cast.  This is the
    sharding/unsharding step; all heavy FLOPs run on device.
  - Device (one expert per core): batched MLP over the expert's gathered
    tokens, padded to capacity C.  bf16 matmuls with fp32 PSUM
    accumulation; SiLU on the scalar engine; combine-weight scaling on the
    vector engine.
  - Host: scatter-add the 8 per-expert outputs into the full [T, H] result.
"""

import time

import numpy as np
import ml_dtypes

from concourse import bass, bacc, tile, mybir
from concourse.bass_utils import run_bass_kernel_spmd

# Problem dims (hardcoded per contract)
T, H, E, I = 4096, 2048, 8, 4096
P = 128          # partitions
KH = H // P      # 16 contraction tiles for MM1
NJ = I // P      # 32 intermediate tiles
HCH = 512        # output hidden chunk
NH = H // HCH    # 4

_BF16 = mybir.dt.bfloat16
_F32 = mybir.dt.float32


def chunk_plan(max_count):
    """Token chunk sizes covering the max per-expert load.  Chunks are the
    matmul moving dim: <=512 (one fp32 PSUM bank), multiples of 128.  At
    most 3 chunks (2*3 MM1 psum banks + 2 MM2 banks = 8); larger loads are
    handled by multiple waves in kernel()."""
    cap = max(P, -(-max_count // P) * P)
    cap = min(cap, 1536)
    plan = [512] * (cap // 512)
    if cap % 512:
        plan.append(cap % 512)
    return tuple(plan)


def build_kernel(plan):
    C = sum(plan)
    nc = bacc.Bacc("TRN2", target_bir_lowering=False, debug=False, num_devices=E)
    xg_d = nc.dram_tensor("xg", [H, C], _BF16, kind="ExternalInput").ap()
    w1t_d = nc.dram_tensor("w1t", [H, 2 * I], _BF16, kind="ExternalInput").ap()
    w2t_d = nc.dram_tensor("w2t", [I, H], _BF16, kind="ExternalInput").ap()
    # combine weights pre-tiled on host: cg[p, q] = weight of token q*128+p
    cg_d = nc.dram_tensor("cg", [P, C // P], _F32, kind="ExternalInput").ap()
    yg_d = nc.dram_tensor("yg", [C, H], _F32, kind="ExternalOutput").ap()

    AF = mybir.ActivationFunctionType

    nchunks = len(plan)
    offs = [sum(plan[:i]) for i in range(nchunks)]
    JG = 2                    # w2 sub-slab j-group
    with tile.TileContext(nc) as tc:
        with (
            tc.tile_pool(name="xp", bufs=1) as xp,
            tc.tile_pool(name="w1p", bufs=2) as w1p,
            tc.tile_pool(name="w2p", bufs=NJ // JG + 2) as w2p,
            tc.tile_pool(name="actp", bufs=1) as actp,
            tc.tile_pool(name="cp", bufs=1) as cp,
            tc.tile_pool(name="sp", bufs=2) as sp,
            tc.tile_pool(name="op", bufs=3) as op,
            tc.tile_pool(name="psA", bufs=1, space="PSUM") as psA,
            tc.tile_pool(name="psB", bufs=2, space="PSUM") as psB,
        ):
            def load_w1_pair(jp):
                n0 = jp * 2 * P
                g = w1p.tile([P, KH, 2 * P], _BF16, tag="w1g",
                             name=f"w1g_{jp}")
                u = w1p.tile([P, KH, 2 * P], _BF16, tag="w1u",
                             name=f"w1u_{jp}")
                nc.sync.dma_start(
                    out=g[:],
                    in_=w1t_d[:, n0:n0 + 2 * P].rearrange(
                        "(k p) n -> p k n", p=P))
                nc.sync.dma_start(
                    out=u[:],
                    in_=w1t_d[:, I + n0:I + n0 + 2 * P].rearrange(
                        "(k p) n -> p k n", p=P))
                return g, u

            # first j-pair's weights BEFORE the bulk xg load: the first
            # matmul needs both, and the DMA queues drain in program order
            gu0 = load_w1_pair(0)

            # gathered tokens, fully resident: one [128, C] tile per h-tile
            xtiles = []
            for k in range(KH):
                xk = xp.tile([P, C], _BF16, tag=f"x{k}")
                nc.sync.dma_start(out=xk[:], in_=xg_d[k * P:(k + 1) * P, :])
                xtiles.append(xk)

            # ---- phase A: h1 = x @ w1.T, act = silu(gate)*up ----
            # j-outer: w1 streamed exactly once; all chunks per PSUM group.
            acts = []
            for jp in range(NJ // 2):
                g, u = gu0 if jp == 0 else load_w1_pair(jp)
                for lj in range(2):
                    j = jp * 2 + lj
                    lsl = slice(lj * P, (lj + 1) * P)
                    pgs = [psA.tile([P, pl], _F32, tag=f"pg{c}",
                                    name=f"pg{c}_{j}")
                           for c, pl in enumerate(plan)]
                    pus = [psA.tile([P, pl], _F32, tag=f"pu{c}",
                                    name=f"pu{c}_{j}")
                           for c, pl in enumerate(plan)]
                    for k in range(KH):
                        for c, pl in enumerate(plan):
                            nc.tensor.matmul(
                                pgs[c][:], g[:, k, lsl],
                                xtiles[k][:, offs[c]:offs[c] + pl],
                                start=(k == 0), stop=(k == KH - 1))
                    for k in range(KH):
                        for c, pl in enumerate(plan):
                            nc.tensor.matmul(
                                pus[c][:], u[:, k, lsl],
                                xtiles[k][:, offs[c]:offs[c] + pl],
                                start=(k == 0), stop=(k == KH - 1))
                    at = actp.tile([P, C], _BF16, tag=f"act{j}")
                    for c, pl in enumerate(plan):
                        st = sp.tile([P, pl], _F32, tag="silu")
                        nc.scalar.activation(st[:], pgs[c][:], AF.Sigmoid)
                        nc.vector.tensor_mul(st[:], st[:], pgs[c][:])
                        nc.vector.tensor_mul(
                            at[:, offs[c]:offs[c] + pl], st[:], pus[c][:])
                    acts.append(at)

            # ---- phase B: y = act @ w2.T, scaled by combine weight ----
            ct = cp.tile([P, C // P], _F32, tag="cg")
            nc.sync.dma_start(out=ct[:], in_=cg_d[:])
            for hc in range(NH):
                w2subs = []
                for jg in range(NJ // JG):
                    w2s = w2p.tile([P, JG, HCH], _BF16, tag="w2")
                    nc.sync.dma_start(
                        out=w2s[:],
                        in_=w2t_d[jg * JG * P:(jg + 1) * JG * P,
                                  hc * HCH:(hc + 1) * HCH].rearrange(
                            "(j p) h -> p j h", p=P))
                    w2subs.append(w2s)
                for tq in range(C // P):
                    po = psB.tile([P, HCH], _F32, tag="po")
                    for j in range(NJ):
                        nc.tensor.matmul(
                            po[:], acts[j][:, tq * P:(tq + 1) * P],
                            w2subs[j // JG][:, j % JG, :],
                            start=(j == 0), stop=(j == NJ - 1))
                    ot = op.tile([P, HCH], _F32, tag="out")
                    nc.vector.tensor_scalar_mul(ot[:], po[:], ct[:, tq:tq + 1])
                    nc.sync.dma_start(
                        out=yg_d[tq * P:(tq + 1) * P, hc * HCH:(hc + 1) * HCH],
                        in_=ot[:])
    nc.compile()
    return nc


_NC_CACHE = {}
LAST_RESULTS = []   # BassKernelResults of each wave of the last kernel() call


def _get_nc(plan):
    if plan not in _NC_CACHE:
        _NC_CACHE[plan] = build_kernel(plan)
    return _NC_CACHE[plan]


def _route(router_logits, top_k):
    """Host routing: stable softmax + top-k (ties broken by lower index,
    matching jax.lax.top_k)."""
    logits = np.asarray(router_logits, dtype=np.float32)
    m = logits.max(axis=-1, keepdims=True)
    p = np.exp(logits - m)
    p /= p.sum(axis=-1, keepdims=True)
    ids = np.argsort(-p, axis=-1, kind="stable")[:, :top_k]   # [T, k]
    gates = np.take_along_axis(p, ids, axis=-1)               # [T, k]
    return ids, gates


def kernel(hidden_states, router_logits, w1, w2, top_k):
    top_k = int(top_k)
    x = np.asarray(hidden_states, dtype=np.float32)
    w1 = np.asarray(w1, dtype=np.float32)
    w2 = np.asarray(w2, dtype=np.float32)
    n_tok, hidden = x.shape
    n_exp = w1.shape[0]
    assert (n_tok, hidden, n_exp) == (T, H, E), "compiled for fixed shapes"

    ids, gates = _route(router_logits, top_k)

    # per-expert token lists
    expert_of = ids.ravel()
    token_of = np.repeat(np.arange(n_tok, dtype=np.int64), top_k)
    gate_of = gates.ravel()
    order = np.argsort(expert_of, kind="stable")
    expert_sorted = expert_of[order]
    token_sorted = token_of[order]
    gate_sorted = gate_of[order]
    counts = np.bincount(expert_sorted, minlength=n_exp)
    starts = np.concatenate([[0], np.cumsum(counts)])

    xT = x.T.astype(ml_dtypes.bfloat16)          # [H, T], contiguous
    w1t = [w1[e].T.astype(ml_dtypes.bfloat16) for e in range(n_exp)]
    w2t = [w2[e].T.astype(ml_dtypes.bfloat16) for e in range(n_exp)]

    plan = chunk_plan(int(counts.max()))
    C = sum(plan)
    nc = _get_nc(plan)
    LAST_RESULTS.clear()
    out = np.zeros((n_tok, hidden), dtype=np.float32)
    done = np.zeros(n_exp, dtype=np.int64)   # tokens dispatched per expert
    while True:
        waves = []
        for e in range(n_exp):
            lo = starts[e] + done[e]
            hi = min(starts[e + 1], lo + C)
            waves.append((lo, hi))
        if all(lo >= hi for lo, hi in waves):
            break
        in_maps = []
        toks_per_e = []
        for e, (lo, hi) in enumerate(waves):
            n_e = hi - lo
            toks = token_sorted[lo:hi]
            toks_per_e.append(toks)
            xg = np.zeros((H, C), dtype=ml_dtypes.bfloat16)
            cg = np.zeros((C,), dtype=np.float32)
            if n_e:
                xg[:, :n_e] = xT[:, toks]
                cg[:n_e] = gate_sorted[lo:hi]
            # pre-tile: cg_t[p, q] = cg[q*128 + p]
            cg = np.ascontiguousarray(cg.reshape(C // P, P).T)
            in_maps.append({"xg": xg, "w1t": w1t[e], "w2t": w2t[e], "cg": cg})
            done[e] += n_e
        try:
            res = run_bass_kernel_spmd(nc, in_maps, list(range(E)))
        except Exception:
            # transient device wedge (e.g. NRT_EXEC_UNIT_UNRECOVERABLE)
            # has been observed to clear on retry
            time.sleep(2)
            res = run_bass_kernel_spmd(nc, in_maps, list(range(E)))
        LAST_RESULTS.append(res)
        for e in range(n_exp):
            toks = toks_per_e[e]
            if len(toks):
                out[toks] += res.results[e]["yg"][:len(toks)]
    return out



# revision 5
# speedup vs baseline: 1.0817x; 1.0817x over previous
"""Expert-parallel MoE GroupedMLP kernel for 8 Trainium2 NeuronCores.

Problem: T=4096 tokens, H=2048 hidden, E=8 experts, I=4096 intermediate,
top_k=2, fp32 reference.

Strategy (balanced two-slot expert-parallel, sharded inside kernel()):
  - Host: softmax + top-k routing.  The token->core assignment is load-
    balanced: every core runs the SAME program with two fixed-size
    single-expert token slots (A, B); which expert each slot serves is
    carried entirely by input data (per-slot weight tensors + gathered
    tokens), so one SPMD program covers an arbitrary expert->core packing.
    Slot sizes are solved from the routing counts by a small exact DP
    (sum A+B ~= ceil(T*top_k/8), vs. ceil-to-128 padding of the naive
    per-expert split).
  - Device: tokens are the matmul moving dim in BOTH matmuls (w1 and w2
    are the stationary operands), so slot sizes need no 128 alignment.
    bf16 matmuls, fp32 PSUM, SiLU on the scalar engine, per-token
    combine-weight scaling on the vector engine.  Weights are host-
    pretiled into the exact SBUF tile layout for contiguous HBM reads.
  - Host: scatter-add the per-slot [H, len] outputs into the [T, H] result.
"""

import time

import numpy as np
import ml_dtypes

from concourse import bass, bacc, tile, mybir
from concourse.bass_utils import run_bass_kernel_spmd

# Problem dims (hardcoded per contract)
T, H, E, I = 4096, 2048, 8, 4096
P = 128          # partitions
KH = H // P      # 16 contraction tiles for MM1
NJ = I // P      # 32 intermediate j-tiles (acts)
SL = I // 256    # 16 w1 slabs of 256 cols per gate/up half
NH = H // P      # 16 output h-tiles
NCORES = 8

_BF16 = mybir.dt.bfloat16
_F32 = mybir.dt.float32


def _split512(n):
    """Split n tokens into matmul chunks of <=512 (PSUM bank limit)."""
    out = []
    while n > 512:
        out.append(512)
        n -= 512
    if n:
        out.append(n)
    return tuple(out)


def solve_slots(counts, ncores=NCORES, bmin=304):
    """Find slot sizes (A, B), A >= B, minimizing A+B such that the expert
    token counts can be covered by <=ncores slots of size A plus <=ncores
    of size B (an expert may use several slots; slots may be left empty).
    B >= bmin when possible: each slot streams its expert's full weights
    (~50 MB), so a slot needs >= ~220 tokens of compute to hide that DMA.
    Feasibility via a bitmask DP over (a-supply, b-supply)."""
    counts = [int(c) for c in counts if c > 0]
    if not counts:
        return 8, 8, []
    total = sum(counts)
    maxn = max(counts)
    s_lo = max(-(-total // ncores), 16)
    sup = ncores + 1

    def feasible(A, B):
        state = np.zeros((sup, sup), dtype=bool)
        state[ncores, ncores] = True
        for n in counts:
            new = np.zeros_like(state)
            for alpha in range(ncores + 1):
                rem = n - alpha * A
                beta = 0 if rem <= 0 else -(-rem // B)
                if beta > ncores:
                    continue
                # using alpha a-slots and beta b-slots
                src = state[alpha:, beta:] if beta else state[alpha:, :]
                if beta:
                    new[:sup - alpha, :sup - beta] |= state[alpha:, beta:]
                else:
                    new[:sup - alpha, :] |= state[alpha:, :]
            state = new
            if not state.any():
                return False
        return True

    def recover(A, B):
        # per-expert (alpha, beta) via DFS with the same minimal-beta rule
        from functools import lru_cache

        @lru_cache(maxsize=None)
        def dfs(e, sa, sb):
            if e == len(counts):
                return ()
            n = counts[e]
            for alpha in range(sa + 1):
                rem = n - alpha * A
                beta = 0 if rem <= 0 else -(-rem // B)
                if beta > sb:
                    continue
                rest = dfs(e + 1, sa - alpha, sb - beta)
                if rest is not None:
                    return ((alpha, beta),) + rest
            return None

        return dfs(0, ncores, ncores)

    for S in range(s_lo, 4 * maxn + 64):
        a_hi = max(-(-S // 2), S - bmin)         # keep B >= bmin if possible
        for A in range(-(-S // 2), a_hi + 1):
            B = S - A
            if B < 8:
                continue
            if feasible(A, B):
                return A, B, list(recover(A, B))
    A = maxn                                     # unreachable
    return A, A, [(1, 1) for _ in counts]


def build_kernel(chunks_a, chunks_b):
    """One SPMD program: two single-expert token slots of fixed sizes.
    Slot weights / tokens / combine-weights are inputs; output per slot is
    y[H, len] (tokens as free dim), already combine-scaled."""
    A = sum(chunks_a)
    B = sum(chunks_b)
    nc = bacc.Bacc("TRN2", target_bir_lowering=False, debug=False,
                   num_devices=NCORES)
    # gathered tokens, [H, len] bf16
    xa_d = nc.dram_tensor("xa", [H, A], _BF16, kind="ExternalInput").ap()
    xb_d = nc.dram_tensor("xb", [H, B], _BF16, kind="ExternalInput").ap()
    # pretiled weights: w1 [32, 128, 4096] = (slab jp, g/u) x p x (k*256+n)
    #                   w2 [16, 128, 4096] = h x p_j x (j*128+hc)
    w1a_d = nc.dram_tensor("w1a", [2 * SL, P, KH * 256], _BF16,
                           kind="ExternalInput").ap()
    w1b_d = nc.dram_tensor("w1b", [2 * SL, P, KH * 256], _BF16,
                           kind="ExternalInput").ap()
    w2a_d = nc.dram_tensor("w2a", [NH, P, NJ * P], _BF16,
                           kind="ExternalInput").ap()
    w2b_d = nc.dram_tensor("w2b", [NH, P, NJ * P], _BF16,
                           kind="ExternalInput").ap()
    # combine weights broadcast across partitions [128, len]
    ca_d = nc.dram_tensor("ca", [P, A], _F32, kind="ExternalInput").ap()
    cb_d = nc.dram_tensor("cb", [P, B], _F32, kind="ExternalInput").ap()
    ya_d = nc.dram_tensor("ya", [H, A], _F32, kind="ExternalOutput").ap()
    yb_d = nc.dram_tensor("yb", [H, B], _F32, kind="ExternalOutput").ap()

    AF = mybir.ActivationFunctionType
    slots = [
        (chunks_a, xa_d, w1a_d, w2a_d, ca_d, ya_d, "a"),
        (chunks_b, xb_d, w1b_d, w2b_d, cb_d, yb_d, "b"),
    ]
    Amax = max(A, B)

    with tile.TileContext(nc) as tc:
        with (
            tc.tile_pool(name="xp", bufs=1) as xp,
            tc.tile_pool(name="w1p", bufs=2) as w1p,
            tc.tile_pool(name="w2p", bufs=3) as w2p,
            tc.tile_pool(name="actp", bufs=1) as actp,
            tc.tile_pool(name="cp", bufs=1) as cp,
            tc.tile_pool(name="sp", bufs=2) as sp,
            tc.tile_pool(name="op", bufs=3) as op,
            tc.tile_pool(name="psA", bufs=1, space="PSUM") as psA,
            tc.tile_pool(name="psB", bufs=2, space="PSUM") as psB,
        ):
            for chunks, x_d, w1_d, w2_d, c_d, y_d, tag in slots:
                C = sum(chunks)
                offs = [sum(chunks[:i]) for i in range(len(chunks))]

                # stream in this slot's tokens (one [128, C] tile per k)
                xtiles = []
                for k in range(KH):
                    xk = xp.tile([P, C], _BF16, tag=f"x{tag}{k}")
                    nc.sync.dma_start(out=xk[:], in_=x_d[k * P:(k + 1) * P, :])
                    xtiles.append(xk)

                # ---- phase A: h1 = x @ w1.T ; act = silu(gate)*up ----
                acts = []
                for jp in range(SL):
                    g = w1p.tile([P, KH * 256], _BF16, tag="w1g",
                                 name=f"w1g_{tag}{jp}")
                    u = w1p.tile([P, KH * 256], _BF16, tag="w1u",
                                 name=f"w1u_{tag}{jp}")
                    nc.sync.dma_start(out=g[:], in_=w1_d[2 * jp])
                    nc.sync.dma_start(out=u[:], in_=w1_d[2 * jp + 1])
                    for lj in range(2):
                        j = jp * 2 + lj
                        pgs = [psA.tile([P, cl], _F32, tag=f"pg{tag}{c}",
                                        name=f"pg{c}_{tag}{j}")
                               for c, cl in enumerate(chunks)]
                        pus = [psA.tile([P, cl], _F32, tag=f"pu{tag}{c}",
                                        name=f"pu{c}_{tag}{j}")
                               for c, cl in enumerate(chunks)]
                        for k in range(KH):
                            ws = slice(k * 256 + lj * P, k * 256 + lj * P + P)
                            for c, cl in enumerate(chunks):
                                nc.tensor.matmul(
                                    pgs[c][:], g[:, ws],
                                    xtiles[k][:, offs[c]:offs[c] + cl],
                                    start=(k == 0), stop=(k == KH - 1))
                        for k in range(KH):
                            ws = slice(k * 256 + lj * P, k * 256 + lj * P + P)
                            for c, cl in enumerate(chunks):
                                nc.tensor.matmul(
                                    pus[c][:], u[:, ws],
                                    xtiles[k][:, offs[c]:offs[c] + cl],
                                    start=(k == 0), stop=(k == KH - 1))
                        at = actp.tile([P, Amax], _BF16, tag=f"act{j}",
                                       name=f"act{j}_{tag}")
                        for c, cl in enumerate(chunks):
                            st = sp.tile([P, cl], _F32, tag="silu")
                            nc.scalar.activation(st[:], pgs[c][:], AF.Sigmoid)
                            nc.vector.tensor_mul(st[:], st[:], pgs[c][:])
                            nc.vector.tensor_mul(
                                at[:, offs[c]:offs[c] + cl], st[:], pus[c][:])
                        acts.append(at)

                # ---- phase B: y = combine * (act @ w2.T) ----
                ct = cp.tile([P, C], _F32, tag=f"c{tag}")
                nc.sync.dma_start(out=ct[:], in_=c_d[:])
                for h in range(NH):
                    wt = w2p.tile([P, NJ * P], _BF16, tag="w2",
                                  name=f"w2_{tag}{h}")
                    nc.sync.dma_start(out=wt[:], in_=w2_d[h])
                    for c, cl in enumerate(chunks):
                        po = psB.tile([P, cl], _F32, tag="po")
                        for j in range(NJ):
                            nc.tensor.matmul(
                                po[:], wt[:, j * P:(j + 1) * P],
                                acts[j][:, offs[c]:offs[c] + cl],
                                start=(j == 0), stop=(j == NJ - 1))
                        ot = op.tile([P, cl], _F32, tag="out")
                        nc.vector.tensor_mul(ot[:], po[:],
                                             ct[:, offs[c]:offs[c] + cl])
                        nc.sync.dma_start(
                            out=y_d[h * P:(h + 1) * P, offs[c]:offs[c] + cl],
                            in_=ot[:])
    nc.compile()
    return nc


_NC_CACHE = {}
_WPACK_CACHE = {}
LAST_RESULTS = []   # BassKernelResults of each wave of the last kernel() call


def _get_nc(chunks_a, chunks_b):
    key = (chunks_a, chunks_b)
    if key not in _NC_CACHE:
        _NC_CACHE[key] = build_kernel(chunks_a, chunks_b)
    return _NC_CACHE[key]


def _pack_weights(w1, w2):
    """Pretile weights into the device tile layout (bf16, contiguous DMA).
    w1 [E, 2I, H] -> [E, 32, 128, 4096]: [e, 2*jp+s, p, k*256+n] =
      w1[e, s*I + jp*256 + n, k*128 + p]        (jp in 0..15, s=gate/up)
    w2 [E, H, I]  -> [E, 16, 128, 4096]: [e, h, p, j*128+hc] =
      w2[e, h*128+hc, j*128+p]
    """
    fp = (w1.shape, w2.shape, w1.ctypes.data, w2.ctypes.data,
          float(w1.flat[0]), float(w2.flat[0]), float(w1.flat[-1]))
    if _WPACK_CACHE.get("fp") == fp:
        return _WPACK_CACHE["w1"], _WPACK_CACHE["w2"]
    # [E, s, jp, n, k, p] -> [E, jp, s, p, k, n]
    w1p = np.ascontiguousarray(
        w1.reshape(E, 2, SL, 256, KH, P).transpose(0, 2, 1, 5, 4, 3)
    ).astype(ml_dtypes.bfloat16).reshape(E, 2 * SL, P, KH * 256)
    # [E, h, hc, j, p] -> [E, h, p, j, hc]
    w2p = np.ascontiguousarray(
        w2.reshape(E, NH, P, NJ, P).transpose(0, 1, 4, 3, 2)
    ).astype(ml_dtypes.bfloat16).reshape(E, NH, P, NJ * P)
    _WPACK_CACHE.update(fp=fp, w1=w1p, w2=w2p)
    return w1p, w2p


def _route(router_logits, top_k):
    """Host routing: stable softmax + top-k (ties broken by lower index,
    matching jax.lax.top_k)."""
    logits = np.asarray(router_logits, dtype=np.float32)
    m = logits.max(axis=-1, keepdims=True)
    p = np.exp(logits - m)
    p /= p.sum(axis=-1, keepdims=True)
    ids = np.argsort(-p, axis=-1, kind="stable")[:, :top_k]   # [T, k]
    gates = np.take_along_axis(p, ids, axis=-1)               # [T, k]
    return ids, gates


def kernel(hidden_states, router_logits, w1, w2, top_k):
    top_k = int(top_k)
    x = np.asarray(hidden_states, dtype=np.float32)
    w1 = np.asarray(w1, dtype=np.float32)
    w2 = np.asarray(w2, dtype=np.float32)
    n_tok, hidden = x.shape
    n_exp = w1.shape[0]
    assert (n_tok, hidden, n_exp) == (T, H, E), "compiled for fixed shapes"

    ids, gates = _route(router_logits, top_k)

    # per-expert token lists (sorted by expert, stable in token order)
    expert_of = ids.ravel()
    token_of = np.repeat(np.arange(n_tok, dtype=np.int64), top_k)
    gate_of = gates.ravel()
    order = np.argsort(expert_of, kind="stable")
    token_sorted = token_of[order]
    gate_sorted = gate_of[order]
    counts = np.bincount(expert_of, minlength=n_exp)
    starts = np.concatenate([[0], np.cumsum(counts)])

    # slot size solve + padding to multiple of 8 for DMA friendliness
    live = [int(c) for c in counts if c > 0]
    live_idx = [e for e in range(n_exp) if counts[e] > 0]
    A, B, pat_live = solve_slots(live)
    pat = [(0, 0)] * n_exp
    for e, ab in zip(live_idx, pat_live):
        pat[e] = ab
    Ap, Bp = -(-A // 8) * 8, -(-B // 8) * 8
    chunks_a, chunks_b = _split512(Ap), _split512(Bp)

    # assign expert segments to slots
    slot_a, slot_b = [], []          # (expert, lo, hi) global sorted idx
    for e in range(n_exp):
        alpha, beta = pat[e]
        lo, hi = int(starts[e]), int(starts[e + 1])
        for _ in range(alpha):
            take = min(A, hi - lo)
            slot_a.append((e, lo, lo + take))
            lo += take
        for _ in range(beta):
            take = min(B, hi - lo)
            slot_b.append((e, lo, lo + take))
            lo += take
        assert lo == hi, "slot solve failed to cover expert"
    while len(slot_a) < NCORES:
        slot_a.append((0, 0, 0))
    while len(slot_b) < NCORES:
        slot_b.append((0, 0, 0))

    xT = x.T.astype(ml_dtypes.bfloat16)          # [H, T], contiguous
    w1pk, w2pk = _pack_weights(w1, w2)

    nc = _get_nc(chunks_a, chunks_b)
    LAST_RESULTS.clear()

    in_maps = []
    for core in range(NCORES):
        m = {}
        for name, (e, lo, hi), L in (("a", slot_a[core], Ap),
                                     ("b", slot_b[core], Bp)):
            n_s = hi - lo
            xg = np.zeros((H, L), dtype=ml_dtypes.bfloat16)
            cg = np.zeros((L,), dtype=np.float32)
            if n_s:
                xg[:, :n_s] = xT[:, token_sorted[lo:hi]]
                cg[:n_s] = gate_sorted[lo:hi]
            m["x" + name] = xg
            m["c" + name] = np.ascontiguousarray(
                np.broadcast_to(cg, (P, L)))
            m["w1" + name] = w1pk[e]
            m["w2" + name] = w2pk[e]
        in_maps.append(m)

    try:
        res = run_bass_kernel_spmd(nc, in_maps, list(range(NCORES)))
    except Exception:
        # transient device wedge has been observed to clear on retry
        time.sleep(2)
        res = run_bass_kernel_spmd(nc, in_maps, list(range(NCORES)))
    LAST_RESULTS.append(res)

    out = np.zeros((n_tok, hidden), dtype=np.float32)
    for core in range(NCORES):
        for name, (e, lo, hi) in (("ya", slot_a[core]), ("yb", slot_b[core])):
            n_s = hi - lo
            if n_s:
                y = res.results[core][name]          # [H, L] f32, scaled
                # tokens are unique within one expert's list -> fancy add ok
                out[token_sorted[lo:hi]] += y[:, :n_s].T
    return out


# revision 12
# speedup vs baseline: 1.0917x; 1.0093x over previous
"""Expert-parallel MoE GroupedMLP kernel for 8 Trainium2 NeuronCores.

Problem: T=4096 tokens, H=2048 hidden, E=8 experts, I=4096 intermediate,
top_k=2, fp32 reference.

Strategy (balanced two-slot expert-parallel, sharded inside kernel()):
  - Host: softmax + top-k routing.  The token->core assignment is load-
    balanced: every core runs the SAME program with two fixed-size
    single-expert token slots (A, B); which expert each slot serves is
    carried entirely by input data (per-slot weight tensors + gathered
    tokens), so one SPMD program covers an arbitrary expert->core packing.
    Slot sizes are solved from the routing counts by a small exact DP
    (sum A+B ~= ceil(T*top_k/8), vs. ceil-to-128 padding of the naive
    per-expert split).
  - Device: tokens are the matmul moving dim in BOTH matmuls (w1 and w2
    are the stationary operands), so slot sizes need no 128 alignment.
    bf16 matmuls, fp32 PSUM, SiLU on the scalar engine, per-token
    combine-weight scaling on the vector engine.  Weights are host-
    pretiled into the exact SBUF tile layout for contiguous HBM reads.
  - Host: scatter-add the per-slot [H, len] outputs into the [T, H] result.
"""

import time

import numpy as np
import ml_dtypes

from concourse import bass, bacc, tile, mybir
from concourse.bass_utils import run_bass_kernel_spmd

# Problem dims (hardcoded per contract)
T, H, E, I = 4096, 2048, 8, 4096
P = 128          # partitions
KH = H // P      # 16 contraction tiles for MM1
NJ = I // P      # 32 intermediate j-tiles (acts)
SL = I // 256    # 16 w1 slabs of 256 cols per gate/up half
NH = H // P      # 16 output h-tiles
NCORES = 8

_BF16 = mybir.dt.bfloat16
_F32 = mybir.dt.float32


def _split512(n):
    """Split n tokens into matmul chunks of <=512 (PSUM bank limit)."""
    out = []
    while n > 512:
        out.append(512)
        n -= 512
    if n:
        out.append(n)
    return tuple(out)


def solve_slots(counts, ncores=NCORES, bmin=304):
    """Find slot sizes (A, B), A >= B, minimizing A+B such that the expert
    token counts can be covered by <=ncores slots of size A plus <=ncores
    of size B (an expert may use several slots; slots may be left empty).
    B >= bmin when possible: each slot streams its expert's full weights
    (~50 MB), so a slot needs >= ~220 tokens of compute to hide that DMA.
    Feasibility via a bitmask DP over (a-supply, b-supply)."""
    counts = [int(c) for c in counts if c > 0]
    if not counts:
        return 8, 8, []
    total = sum(counts)
    maxn = max(counts)
    s_lo = max(-(-total // ncores), 16)
    sup = ncores + 1

    def feasible(A, B):
        state = np.zeros((sup, sup), dtype=bool)
        state[ncores, ncores] = True
        for n in counts:
            new = np.zeros_like(state)
            for alpha in range(ncores + 1):
                rem = n - alpha * A
                beta = 0 if rem <= 0 else -(-rem // B)
                if beta > ncores:
                    continue
                # using alpha a-slots and beta b-slots
                src = state[alpha:, beta:] if beta else state[alpha:, :]
                if beta:
                    new[:sup - alpha, :sup - beta] |= state[alpha:, beta:]
                else:
                    new[:sup - alpha, :] |= state[alpha:, :]
            state = new
            if not state.any():
                return False
        return True

    def recover(A, B):
        # per-expert (alpha, beta) via DFS with the same minimal-beta rule
        from functools import lru_cache

        @lru_cache(maxsize=None)
        def dfs(e, sa, sb):
            if e == len(counts):
                return ()
            n = counts[e]
            for alpha in range(sa + 1):
                rem = n - alpha * A
                beta = 0 if rem <= 0 else -(-rem // B)
                if beta > sb:
                    continue
                rest = dfs(e + 1, sa - alpha, sb - beta)
                if rest is not None:
                    return ((alpha, beta),) + rest
            return None

        return dfs(0, ncores, ncores)

    for S in range(s_lo, 4 * maxn + 64):
        a_hi = max(-(-S // 2), S - bmin)         # keep B >= bmin if possible
        for A in range(-(-S // 2), a_hi + 1):
            B = S - A
            if B < 8:
                continue
            if feasible(A, B):
                return A, B, list(recover(A, B))
    A = maxn                                     # unreachable
    return A, A, [(1, 1) for _ in counts]


def build_kernel(chunks_a, chunks_b):
    """One SPMD program: two single-expert token slots of fixed sizes.
    Slot weights / tokens / combine-weights are inputs; output per slot is
    y[H, len] (tokens as free dim), already combine-scaled."""
    A = sum(chunks_a)
    B = sum(chunks_b)
    nc = bacc.Bacc("TRN2", target_bir_lowering=False, debug=False,
                   num_devices=NCORES)
    # gathered tokens, k-pair-packed [KH//2, 128, 2*len] bf16 (2KB+ lines)
    xa_d = nc.dram_tensor("xa", [KH // 2, P, 2 * A], _BF16,
                          kind="ExternalInput").ap()
    xb_d = nc.dram_tensor("xb", [KH // 2, P, 2 * B], _BF16,
                          kind="ExternalInput").ap()
    # pretiled weights: w1 [32, 128, 4096] = (slab jp, g/u) x p x (k*256+n)
    #                   w2 [16, 128, 4096] = h x p_j x (j*128+hc)
    w1a_d = nc.dram_tensor("w1a", [2 * SL, P, KH * 256], _BF16,
                           kind="ExternalInput").ap()
    w1b_d = nc.dram_tensor("w1b", [2 * SL, P, KH * 256], _BF16,
                           kind="ExternalInput").ap()
    w2a_d = nc.dram_tensor("w2a", [NH, P, NJ * P], _BF16,
                           kind="ExternalInput").ap()
    w2b_d = nc.dram_tensor("w2b", [NH, P, NJ * P], _BF16,
                           kind="ExternalInput").ap()
    # combine weights broadcast across partitions [128, len]
    ca_d = nc.dram_tensor("ca", [P, A], _F32, kind="ExternalInput").ap()
    cb_d = nc.dram_tensor("cb", [P, B], _F32, kind="ExternalInput").ap()
    ya_d = nc.dram_tensor("ya", [H, A], _F32, kind="ExternalOutput").ap()
    yb_d = nc.dram_tensor("yb", [H, B], _F32, kind="ExternalOutput").ap()

    AF = mybir.ActivationFunctionType
    slots = [
        (chunks_a, xa_d, w1a_d, w2a_d, ca_d, ya_d, "a"),
        (chunks_b, xb_d, w1b_d, w2b_d, cb_d, yb_d, "b"),
    ]
    Amax = max(A, B)

    with tile.TileContext(nc) as tc:
        with (
            tc.tile_pool(name="xp", bufs=1) as xp,
            tc.tile_pool(name="w1p", bufs=2) as w1p,
            tc.tile_pool(name="w2p", bufs=3) as w2p,
            tc.tile_pool(name="actp", bufs=1) as actp,
            tc.tile_pool(name="cp", bufs=1) as cp,
            tc.tile_pool(name="sp", bufs=2) as sp,
            tc.tile_pool(name="op", bufs=3) as op,
            tc.tile_pool(name="psA", bufs=1, space="PSUM") as psA,
            tc.tile_pool(name="psB", bufs=2, space="PSUM") as psB,
        ):
            for chunks, x_d, w1_d, w2_d, c_d, y_d, tag in slots:
                C = sum(chunks)
                offs = [sum(chunks[:i]) for i in range(len(chunks))]

                def load_slab(w1_d, jp, tag):
                    g = w1p.tile([P, KH * 256], _BF16, tag="w1g",
                                 name=f"w1g_{tag}{jp}")
                    u = w1p.tile([P, KH * 256], _BF16, tag="w1u",
                                 name=f"w1u_{tag}{jp}")
                    nc.sync.dma_start(out=g[:], in_=w1_d[2 * jp])
                    nc.sync.dma_start(out=u[:], in_=w1_d[2 * jp + 1])
                    return g, u

                # first slab ahead of the token stream: the first matmul
                # needs both, and each queue drains in program order
                gu0 = load_slab(w1_d, 0, tag)

                # this slot's tokens: one [128, 2C] tile per k-pair, on the
                # scalar HWDGE queue (parallel with the sync weight queue)
                xtiles = []            # (tile, column base) per k-tile
                for kk in range(KH // 2):
                    xk = xp.tile([P, 2 * C], _BF16, tag=f"x{tag}{kk}")
                    nc.scalar.dma_start(out=xk[:], in_=x_d[kk])
                    xtiles.append((xk, 0))
                    xtiles.append((xk, C))

                # ---- phase A: h1 = x @ w1.T ; act = silu(gate)*up ----
                acts = []
                for jp in range(SL):
                    g, u = gu0 if jp == 0 else load_slab(w1_d, jp, tag)
                    for lj in range(2):
                        j = jp * 2 + lj
                        pgs = [psA.tile([P, cl], _F32, tag=f"pg{tag}{c}",
                                        name=f"pg{c}_{tag}{j}")
                               for c, cl in enumerate(chunks)]
                        pus = [psA.tile([P, cl], _F32, tag=f"pu{tag}{c}",
                                        name=f"pu{c}_{tag}{j}")
                               for c, cl in enumerate(chunks)]
                        for k in range(KH):
                            ws = slice(k * 256 + lj * P, k * 256 + lj * P + P)
                            xt, xb = xtiles[k]
                            for c, cl in enumerate(chunks):
                                o = xb + offs[c]
                                nc.tensor.matmul(
                                    pgs[c][:], g[:, ws], xt[:, o:o + cl],
                                    start=(k == 0), stop=(k == KH - 1))
                        for k in range(KH):
                            ws = slice(k * 256 + lj * P, k * 256 + lj * P + P)
                            xt, xb = xtiles[k]
                            for c, cl in enumerate(chunks):
                                o = xb + offs[c]
                                nc.tensor.matmul(
                                    pus[c][:], u[:, ws], xt[:, o:o + cl],
                                    start=(k == 0), stop=(k == KH - 1))
                        at = actp.tile([P, Amax], _BF16, tag=f"act{j}",
                                       name=f"act{j}_{tag}")
                        for c, cl in enumerate(chunks):
                            st = sp.tile([P, cl], _F32, tag="silu")
                            nc.scalar.activation(st[:], pgs[c][:], AF.Sigmoid)
                            nc.vector.tensor_mul(st[:], st[:], pgs[c][:])
                            nc.vector.tensor_mul(
                                at[:, offs[c]:offs[c] + cl], st[:], pus[c][:])
                        acts.append(at)

                # ---- phase B: y = combine * (act @ w2.T) ----
                ct = cp.tile([P, C], _F32, tag=f"c{tag}")
                nc.scalar.dma_start(out=ct[:], in_=c_d[:])
                for h in range(NH):
                    wt = w2p.tile([P, NJ * P], _BF16, tag="w2",
                                  name=f"w2_{tag}{h}")
                    nc.sync.dma_start(out=wt[:], in_=w2_d[h])
                    for c, cl in enumerate(chunks):
                        po = psB.tile([P, cl], _F32, tag="po")
                        for j in range(NJ):
                            nc.tensor.matmul(
                                po[:], wt[:, j * P:(j + 1) * P],
                                acts[j][:, offs[c]:offs[c] + cl],
                                start=(j == 0), stop=(j == NJ - 1))
                        ot = op.tile([P, cl], _F32, tag="out")
                        nc.vector.tensor_mul(ot[:], po[:],
                                             ct[:, offs[c]:offs[c] + cl])
                        nc.scalar.dma_start(
                            out=y_d[h * P:(h + 1) * P, offs[c]:offs[c] + cl],
                            in_=ot[:])
    nc.compile()
    return nc


_NC_CACHE = {}
_WPACK_CACHE = {}
LAST_RESULTS = []   # BassKernelResults of each wave of the last kernel() call


def _get_nc(chunks_a, chunks_b):
    key = (chunks_a, chunks_b)
    if key not in _NC_CACHE:
        _NC_CACHE[key] = build_kernel(chunks_a, chunks_b)
    return _NC_CACHE[key]


def _pack_weights(w1, w2):
    """Pretile weights into the device tile layout (bf16, contiguous DMA).
    w1 [E, 2I, H] -> [E, 32, 128, 4096]: [e, 2*jp+s, p, k*256+n] =
      w1[e, s*I + jp*256 + n, k*128 + p]        (jp in 0..15, s=gate/up)
    w2 [E, H, I]  -> [E, 16, 128, 4096]: [e, h, p, j*128+hc] =
      w2[e, h*128+hc, j*128+p]
    """
    fp = (w1.shape, w2.shape, w1.ctypes.data, w2.ctypes.data,
          float(w1.flat[0]), float(w2.flat[0]), float(w1.flat[-1]))
    if _WPACK_CACHE.get("fp") == fp:
        return _WPACK_CACHE["w1"], _WPACK_CACHE["w2"]
    # [E, s, jp, n, k, p] -> [E, jp, s, p, k, n]
    w1p = np.ascontiguousarray(
        w1.reshape(E, 2, SL, 256, KH, P).transpose(0, 2, 1, 5, 4, 3)
    ).astype(ml_dtypes.bfloat16).reshape(E, 2 * SL, P, KH * 256)
    # [E, h, hc, j, p] -> [E, h, p, j, hc]
    w2p = np.ascontiguousarray(
        w2.reshape(E, NH, P, NJ, P).transpose(0, 1, 4, 3, 2)
    ).astype(ml_dtypes.bfloat16).reshape(E, NH, P, NJ * P)
    _WPACK_CACHE.update(fp=fp, w1=w1p, w2=w2p)
    return w1p, w2p


def _route(router_logits, top_k):
    """Host routing: stable softmax + top-k (ties broken by lower index,
    matching jax.lax.top_k)."""
    logits = np.asarray(router_logits, dtype=np.float32)
    m = logits.max(axis=-1, keepdims=True)
    p = np.exp(logits - m)
    p /= p.sum(axis=-1, keepdims=True)
    ids = np.argsort(-p, axis=-1, kind="stable")[:, :top_k]   # [T, k]
    gates = np.take_along_axis(p, ids, axis=-1)               # [T, k]
    return ids, gates


def kernel(hidden_states, router_logits, w1, w2, top_k):
    top_k = int(top_k)
    x = np.asarray(hidden_states, dtype=np.float32)
    w1 = np.asarray(w1, dtype=np.float32)
    w2 = np.asarray(w2, dtype=np.float32)
    n_tok, hidden = x.shape
    n_exp = w1.shape[0]
    assert (n_tok, hidden, n_exp) == (T, H, E), "compiled for fixed shapes"

    ids, gates = _route(router_logits, top_k)

    # per-expert token lists (sorted by expert, stable in token order)
    expert_of = ids.ravel()
    token_of = np.repeat(np.arange(n_tok, dtype=np.int64), top_k)
    gate_of = gates.ravel()
    order = np.argsort(expert_of, kind="stable")
    token_sorted = token_of[order]
    gate_sorted = gate_of[order]
    counts = np.bincount(expert_of, minlength=n_exp)
    starts = np.concatenate([[0], np.cumsum(counts)])

    # slot size solve + padding to multiple of 8 for DMA friendliness
    live = [int(c) for c in counts if c > 0]
    live_idx = [e for e in range(n_exp) if counts[e] > 0]
    A, B, pat_live = solve_slots(live)
    pat = [(0, 0)] * n_exp
    for e, ab in zip(live_idx, pat_live):
        pat[e] = ab
    Ap, Bp = -(-A // 8) * 8, -(-B // 8) * 8
    chunks_a, chunks_b = _split512(Ap), _split512(Bp)

    # assign expert segments to slots
    slot_a, slot_b = [], []          # (expert, lo, hi) global sorted idx
    for e in range(n_exp):
        alpha, beta = pat[e]
        lo, hi = int(starts[e]), int(starts[e + 1])
        for _ in range(alpha):
            take = min(A, hi - lo)
            slot_a.append((e, lo, lo + take))
            lo += take
        for _ in range(beta):
            take = min(B, hi - lo)
            slot_b.append((e, lo, lo + take))
            lo += take
        assert lo == hi, "slot solve failed to cover expert"
    while len(slot_a) < NCORES:
        slot_a.append((0, 0, 0))
    while len(slot_b) < NCORES:
        slot_b.append((0, 0, 0))

    xT = x.T.astype(ml_dtypes.bfloat16)          # [H, T], contiguous
    w1pk, w2pk = _pack_weights(w1, w2)

    nc = _get_nc(chunks_a, chunks_b)
    LAST_RESULTS.clear()

    in_maps = []
    for core in range(NCORES):
        m = {}
        for name, (e, lo, hi), L in (("a", slot_a[core], Ap),
                                     ("b", slot_b[core], Bp)):
            n_s = hi - lo
            xg = np.zeros((H, L), dtype=ml_dtypes.bfloat16)
            cg = np.zeros((L,), dtype=np.float32)
            if n_s:
                xg[:, :n_s] = xT[:, token_sorted[lo:hi]]
                cg[:n_s] = gate_sorted[lo:hi]
            # pack k-tile pairs: [KH//2, 128, 2L] with row = [k-even|k-odd]
            m["x" + name] = np.ascontiguousarray(
                xg.reshape(KH // 2, 2, P, L).transpose(0, 2, 1, 3)
            ).reshape(KH // 2, P, 2 * L)
            m["c" + name] = np.ascontiguousarray(
                np.broadcast_to(cg, (P, L)))
            m["w1" + name] = w1pk[e]
            m["w2" + name] = w2pk[e]
        in_maps.append(m)

    try:
        res = run_bass_kernel_spmd(nc, in_maps, list(range(NCORES)))
    except Exception:
        # transient device wedge has been observed to clear on retry
        time.sleep(2)
        res = run_bass_kernel_spmd(nc, in_maps, list(range(NCORES)))
    LAST_RESULTS.append(res)

    out = np.zeros((n_tok, hidden), dtype=np.float32)
    for core in range(NCORES):
        for name, (e, lo, hi) in (("ya", slot_a[core]), ("yb", slot_b[core])):
            n_s = hi - lo
            if n_s:
                y = res.results[core][name]          # [H, L] f32, scaled
                # tokens are unique within one expert's list -> fancy add ok
                out[token_sorted[lo:hi]] += y[:, :n_s].T
    return out


# revision 13
# speedup vs baseline: 1.1122x; 1.0188x over previous
"""Expert-parallel MoE GroupedMLP kernel for 8 Trainium2 NeuronCores.

Problem: T=4096 tokens, H=2048 hidden, E=8 experts, I=4096 intermediate,
top_k=2, fp32 reference.

Strategy (balanced multi-slot expert-parallel, sharded inside kernel()):
  - Host: softmax + top-k routing.  The token->core assignment is load-
    balanced: every core runs the SAME program with k (2 or 3) fixed-size
    single-expert token slots; which expert each slot serves is carried
    entirely by input data (per-slot weight tensors + gathered tokens), so
    one SPMD program covers an arbitrary expert->core packing.  Slot sizes
    are solved from the routing counts by a small exact DP; slots are kept
    >= ~300 tokens so each slot's full-weight stream (~50 MB) stays hidden
    under its compute.
  - Device: tokens are the matmul moving dim in BOTH matmuls (w1 and w2
    are the stationary operands), so slot sizes need no 128 alignment.
    bf16 matmuls, fp32 PSUM, SiLU on the scalar engine, per-token
    combine-weight scaling on the vector engine.  Weights are host-
    pretiled into the exact SBUF tile layout for contiguous HBM reads;
    weight streams ride the sync HWDGE queue, tokens/combine/outputs the
    scalar HWDGE queue.
  - Host: scatter-add the per-slot [H, len] outputs into the [T, H] result.
"""

import time

import numpy as np
import ml_dtypes

from concourse import bass, bacc, tile, mybir
from concourse.bass_utils import run_bass_kernel_spmd

# Problem dims (hardcoded per contract)
T, H, E, I = 4096, 2048, 8, 4096
P = 128          # partitions
KH = H // P      # 16 contraction tiles for MM1
NJ = I // P      # 32 intermediate j-tiles (acts)
SL = I // 256    # 16 w1 slabs of 256 cols per gate/up half
NH = H // P      # 16 output h-tiles
NCORES = 8
BMIN = 304       # min slot size: hide the ~50MB/slot weight stream

_BF16 = mybir.dt.bfloat16
_F32 = mybir.dt.float32


def _split512(n):
    """Split n tokens into matmul chunks of <=512 (PSUM bank limit)."""
    out = []
    while n > 512:
        out.append(512)
        n -= 512
    if n:
        out.append(n)
    return tuple(out)


def _pad4(n):
    return -(-n // 4) * 4


def solve_slots2(counts, ncores=NCORES, bmin=BMIN, cap=None):
    """Slot sizes (A, B), A >= B, minimizing A+B such that the expert
    token counts are covered by <=ncores slots of each size (an expert may
    use several slots; slots may be left empty).  Bitmask DP feasibility."""
    counts = [int(c) for c in counts]
    total = sum(counts)
    maxn = max(counts)
    s_lo = max(-(-total // ncores), 16)
    sup = ncores + 1

    def feasible(A, B):
        state = np.zeros((sup, sup), dtype=bool)
        state[ncores, ncores] = True
        for n in counts:
            new = np.zeros_like(state)
            for alpha in range(ncores + 1):
                rem = n - alpha * A
                beta = 0 if rem <= 0 else -(-rem // B)
                if beta > ncores:
                    continue
                if beta:
                    new[:sup - alpha or None, :sup - beta] |= \
                        state[alpha:, beta:]
                else:
                    new[:sup - alpha or None, :] |= state[alpha:, :]
                if rem <= 0:
                    break
            state = new
            if not state.any():
                return False
        return True

    def recover(A, B):
        from functools import lru_cache

        @lru_cache(maxsize=None)
        def dfs(e, sa, sb):
            if e == len(counts):
                return ()
            n = counts[e]
            for alpha in range(sa + 1):
                rem = n - alpha * A
                beta = 0 if rem <= 0 else -(-rem // B)
                if beta > sb:
                    continue
                rest = dfs(e + 1, sa - alpha, sb - beta)
                if rest is not None:
                    return ((alpha, beta),) + rest
            return None

        return dfs(0, ncores, ncores)

    for S in range(s_lo, 4 * maxn + 64):
        a_hi = max(-(-S // 2), S - bmin)         # keep B >= bmin if possible
        for A in range(-(-S // 2), a_hi + 1):
            B = S - A
            if B < 8 or (cap and A > cap):
                continue
            if feasible(A, B):
                pat = recover(A, B)
                return (A, B), [tuple(p) for p in pat]
    return None


def solve_slots3(counts, ncores=NCORES, bmin=BMIN, s_max=None,
                 budget_s=45.0):
    """Three slot sizes (a >= b >= c >= bmin) minimizing a+b+c, same cover
    rules.  Returns None if infeasible within bounds/budget."""
    counts = [int(c) for c in counts]
    total = sum(counts)
    maxn = max(counts)
    s_lo = max(-(-total // ncores), 3 * bmin)
    if s_max is None:
        s_max = 4 * maxn
    sup = ncores + 1
    t0 = time.time()

    def feasible(a, b, c):
        state = np.zeros((sup, sup, sup), dtype=bool)
        state[ncores, ncores, ncores] = True
        for n in counts:
            new = np.zeros_like(state)
            for al in range(ncores + 1):
                r1 = n - al * a
                for be in range(ncores + 1):
                    r2 = r1 - be * b
                    ga = 0 if r2 <= 0 else -(-r2 // c)
                    if ga > ncores:
                        continue
                    if ga:
                        new[:sup - al or None, :sup - be or None,
                            :sup - ga] |= state[al:, be:, ga:]
                    elif be:
                        new[:sup - al or None, :sup - be, :] |= \
                            state[al:, be:, :]
                    else:
                        new[:sup - al or None, :, :] |= state[al:, :, :]
                    if r1 <= 0:
                        break
                state_any = True
            state = new
            if not state.any():
                return False
        return True

    def recover(a, b, c):
        from functools import lru_cache

        @lru_cache(maxsize=None)
        def dfs(e, sa, sb, sc):
            if e == len(counts):
                return ()
            n = counts[e]
            for al in range(sa + 1):
                r1 = n - al * a
                for be in range(sb + 1):
                    r2 = r1 - be * b
                    ga = 0 if r2 <= 0 else -(-r2 // c)
                    if ga > sc:
                        continue
                    rest = dfs(e + 1, sa - al, sb - be, sc - ga)
                    if rest is not None:
                        return ((al, be, ga),) + rest
                    if r1 <= 0:
                        break
            return None

        return dfs(0, ncores, ncores, ncores)

    for S in range(s_lo, s_max):
        for a in range(-(-S // 3), S - 2 * bmin + 1):
            for b in range(max(bmin, -(-(S - a) // 2)),
                           min(a, S - a - bmin) + 1):
                c = S - a - b
                if c < bmin or c > b:
                    continue
                if time.time() - t0 > budget_s:
                    return None
                if feasible(a, b, c):
                    pat = recover(a, b, c)
                    if pat is not None:
                        return (a, b, c), [tuple(p) for p in pat]
    return None


def solve_slots(counts):
    """Best slot plan: try 3 slots, fall back to 2.  Returns (sizes, pat)
    with sizes already padded to a multiple of 4."""
    counts = [int(c) for c in counts]
    r2 = solve_slots2(counts)
    s2 = sum(r2[0]) if r2 else 1 << 30
    r3 = solve_slots3(counts, s_max=min(s2, 4 * max(counts)))
    best = None
    if r3 is not None and sum(_pad4(s) for s in r3[0]) < \
            (sum(_pad4(s) for s in r2[0]) if r2 else 1 << 30):
        best = r3
    elif r2 is not None:
        best = r2
    else:
        # trivial always-feasible fallback: one big slot per expert
        A = max(counts)
        best = ((A, A), [(1, 1) for _ in counts])
    sizes, pat = best
    padded = tuple(_pad4(s) for s in sizes)
    # PSUM budget: 2 psum tiles per chunk (+2 for phase B) must fit 8 banks
    while sum(len(_split512(p)) for p in padded) * 2 + 2 > 8:
        # extremely skewed routing: cap slot sizes at 512 and re-solve
        r2 = solve_slots2(counts, cap=512)
        sizes, pat = r2
        padded = tuple(_pad4(s) for s in sizes)
        break
    return sizes, padded, pat


def build_kernel(slot_chunks):
    """One SPMD program: len(slot_chunks) single-expert token slots of
    fixed sizes.  Slot weights / tokens / combine-weights are inputs;
    output per slot is y[H, len] (tokens free dim), combine-scaled."""
    nslots = len(slot_chunks)
    lens = [sum(ch) for ch in slot_chunks]
    Amax = max(lens)
    nc = bacc.Bacc("TRN2", target_bir_lowering=False, debug=False,
                   num_devices=NCORES)
    dts = []
    for i, L in enumerate(lens):
        dts.append((
            nc.dram_tensor(f"x{i}", [KH // 2, P, 2 * L], _BF16,
                           kind="ExternalInput").ap(),
            nc.dram_tensor(f"w1_{i}", [2 * SL, P, KH * 256], _BF16,
                           kind="ExternalInput").ap(),
            nc.dram_tensor(f"w2_{i}", [NH, P, NJ * P], _BF16,
                           kind="ExternalInput").ap(),
            nc.dram_tensor(f"c{i}", [P, L], _F32,
                           kind="ExternalInput").ap(),
            nc.dram_tensor(f"y{i}", [H, L], _F32,
                           kind="ExternalOutput").ap(),
        ))

    AF = mybir.ActivationFunctionType

    with tile.TileContext(nc) as tc:
        with (
            tc.tile_pool(name="xp", bufs=1) as xp,
            tc.tile_pool(name="w1p", bufs=2) as w1p,
            tc.tile_pool(name="w2p", bufs=3) as w2p,
            tc.tile_pool(name="actp", bufs=1) as actp,
            tc.tile_pool(name="cp", bufs=1) as cp,
            tc.tile_pool(name="sp", bufs=2) as sp,
            tc.tile_pool(name="op", bufs=3) as op,
            tc.tile_pool(name="psA", bufs=1, space="PSUM") as psA,
            tc.tile_pool(name="psB", bufs=2, space="PSUM") as psB,
        ):
            for si, chunks in enumerate(slot_chunks):
                x_d, w1_d, w2_d, c_d, y_d = dts[si]
                C = lens[si]
                offs = [sum(chunks[:i]) for i in range(len(chunks))]

                def load_slab(jp):
                    g = w1p.tile([P, KH * 256], _BF16, tag="w1g",
                                 name=f"w1g_{si}_{jp}")
                    u = w1p.tile([P, KH * 256], _BF16, tag="w1u",
                                 name=f"w1u_{si}_{jp}")
                    nc.sync.dma_start(out=g[:], in_=w1_d[2 * jp])
                    nc.sync.dma_start(out=u[:], in_=w1_d[2 * jp + 1])
                    return g, u

                # first slab ahead of the token stream: the first matmul
                # needs both, and each queue drains in program order
                gu0 = load_slab(0)

                # this slot's tokens: one [128, 2C] tile per k-pair, on the
                # scalar HWDGE queue (parallel with the sync weight queue)
                xtiles = []            # (tile, column base) per k-tile
                for kk in range(KH // 2):
                    xk = xp.tile([P, 2 * C], _BF16, tag=f"x{si}_{kk}")
                    nc.scalar.dma_start(out=xk[:], in_=x_d[kk])
                    xtiles.append((xk, 0))
                    xtiles.append((xk, C))

                # ---- phase A: h1 = x @ w1.T ; act = silu(gate)*up ----
                acts = []
                for jp in range(SL):
                    g, u = gu0 if jp == 0 else load_slab(jp)
                    for lj in range(2):
                        j = jp * 2 + lj
                        pgs = [psA.tile([P, cl], _F32, tag=f"pg{si}_{c}",
                                        name=f"pg{c}_{si}_{j}")
                               for c, cl in enumerate(chunks)]
                        pus = [psA.tile([P, cl], _F32, tag=f"pu{si}_{c}",
                                        name=f"pu{c}_{si}_{j}")
                               for c, cl in enumerate(chunks)]
                        for k in range(KH):
                            ws = slice(k * 256 + lj * P, k * 256 + lj * P + P)
                            xt, xb = xtiles[k]
                            for c, cl in enumerate(chunks):
                                o = xb + offs[c]
                                nc.tensor.matmul(
                                    pgs[c][:], g[:, ws], xt[:, o:o + cl],
                                    start=(k == 0), stop=(k == KH - 1))
                        for k in range(KH):
                            ws = slice(k * 256 + lj * P, k * 256 + lj * P + P)
                            xt, xb = xtiles[k]
                            for c, cl in enumerate(chunks):
                                o = xb + offs[c]
                                nc.tensor.matmul(
                                    pus[c][:], u[:, ws], xt[:, o:o + cl],
                                    start=(k == 0), stop=(k == KH - 1))
                        at = actp.tile([P, Amax], _BF16, tag=f"act{j}",
                                       name=f"act{j}_{si}")
                        for c, cl in enumerate(chunks):
                            st = sp.tile([P, cl], _F32, tag="silu")
                            nc.scalar.activation(st[:], pgs[c][:], AF.Sigmoid)
                            nc.vector.tensor_mul(st[:], st[:], pgs[c][:])
                            nc.vector.tensor_mul(
                                at[:, offs[c]:offs[c] + cl], st[:], pus[c][:])
                        acts.append(at)

                # ---- phase B: y = combine * (act @ w2.T) ----
                ct = cp.tile([P, C], _F32, tag=f"c{si}")
                nc.scalar.dma_start(out=ct[:], in_=c_d[:])
                for h in range(NH):
                    wt = w2p.tile([P, NJ * P], _BF16, tag="w2",
                                  name=f"w2_{si}_{h}")
                    nc.sync.dma_start(out=wt[:], in_=w2_d[h])
                    for c, cl in enumerate(chunks):
                        po = psB.tile([P, cl], _F32, tag="po")
                        for j in range(NJ):
                            nc.tensor.matmul(
                                po[:], wt[:, j * P:(j + 1) * P],
                                acts[j][:, offs[c]:offs[c] + cl],
                                start=(j == 0), stop=(j == NJ - 1))
                        ot = op.tile([P, cl], _F32, tag="out")
                        nc.vector.tensor_mul(ot[:], po[:],
                                             ct[:, offs[c]:offs[c] + cl])
                        nc.scalar.dma_start(
                            out=y_d[h * P:(h + 1) * P, offs[c]:offs[c] + cl],
                            in_=ot[:])
    nc.compile()
    return nc


_NC_CACHE = {}
_WPACK_CACHE = {}
LAST_RESULTS = []   # BassKernelResults of each wave of the last kernel() call


def _get_nc(slot_chunks):
    if slot_chunks not in _NC_CACHE:
        _NC_CACHE[slot_chunks] = build_kernel(slot_chunks)
    return _NC_CACHE[slot_chunks]


def _pack_weights(w1, w2):
    """Pretile weights into the device tile layout (bf16, contiguous DMA).
    w1 [E, 2I, H] -> [E, 32, 128, 4096]: [e, 2*jp+s, p, k*256+n] =
      w1[e, s*I + jp*256 + n, k*128 + p]        (jp in 0..15, s=gate/up)
    w2 [E, H, I]  -> [E, 16, 128, 4096]: [e, h, p, j*128+hc] =
      w2[e, h*128+hc, j*128+p]
    """
    fp = (w1.shape, w2.shape, w1.ctypes.data, w2.ctypes.data,
          float(w1.flat[0]), float(w2.flat[0]), float(w1.flat[-1]))
    if _WPACK_CACHE.get("fp") == fp:
        return _WPACK_CACHE["w1"], _WPACK_CACHE["w2"]
    w1p = np.ascontiguousarray(
        w1.reshape(E, 2, SL, 256, KH, P).transpose(0, 2, 1, 5, 4, 3)
    ).astype(ml_dtypes.bfloat16).reshape(E, 2 * SL, P, KH * 256)
    w2p = np.ascontiguousarray(
        w2.reshape(E, NH, P, NJ, P).transpose(0, 1, 4, 3, 2)
    ).astype(ml_dtypes.bfloat16).reshape(E, NH, P, NJ * P)
    _WPACK_CACHE.update(fp=fp, w1=w1p, w2=w2p)
    return w1p, w2p


def _route(router_logits, top_k):
    """Host routing: stable softmax + top-k (ties broken by lower index,
    matching jax.lax.top_k)."""
    logits = np.asarray(router_logits, dtype=np.float32)
    m = logits.max(axis=-1, keepdims=True)
    p = np.exp(logits - m)
    p /= p.sum(axis=-1, keepdims=True)
    ids = np.argsort(-p, axis=-1, kind="stable")[:, :top_k]   # [T, k]
    gates = np.take_along_axis(p, ids, axis=-1)               # [T, k]
    return ids, gates


def kernel(hidden_states, router_logits, w1, w2, top_k):
    top_k = int(top_k)
    x = np.asarray(hidden_states, dtype=np.float32)
    w1 = np.asarray(w1, dtype=np.float32)
    w2 = np.asarray(w2, dtype=np.float32)
    n_tok, hidden = x.shape
    n_exp = w1.shape[0]
    assert (n_tok, hidden, n_exp) == (T, H, E), "compiled for fixed shapes"

    ids, gates = _route(router_logits, top_k)

    # per-expert token lists (sorted by expert, stable in token order)
    expert_of = ids.ravel()
    token_of = np.repeat(np.arange(n_tok, dtype=np.int64), top_k)
    gate_of = gates.ravel()
    order = np.argsort(expert_of, kind="stable")
    token_sorted = token_of[order]
    gate_sorted = gate_of[order]
    counts = np.bincount(expert_of, minlength=n_exp)
    starts = np.concatenate([[0], np.cumsum(counts)])

    live = [int(c) for c in counts if c > 0]
    live_idx = [e for e in range(n_exp) if counts[e] > 0]
    sizes, padded, pat_live = solve_slots(live)
    nslots = len(sizes)
    pat = [(0,) * nslots] * n_exp
    for e, ab in zip(live_idx, pat_live):
        pat[e] = ab
    slot_chunks = tuple(_split512(p) for p in padded)

    # assign expert segments to slots: slot_lists[i] = [(e, lo, hi), ...]
    slot_lists = [[] for _ in range(nslots)]
    for e in range(n_exp):
        lo, hi = int(starts[e]), int(starts[e + 1])
        for i in range(nslots):
            for _ in range(pat[e][i]):
                take = min(sizes[i], hi - lo)
                slot_lists[i].append((e, lo, lo + take))
                lo += take
        assert lo == hi, "slot solve failed to cover expert"
    for sl in slot_lists:
        while len(sl) < NCORES:
            sl.append((0, 0, 0))

    xT = x.T.astype(ml_dtypes.bfloat16)          # [H, T], contiguous
    w1pk, w2pk = _pack_weights(w1, w2)

    nc = _get_nc(slot_chunks)
    LAST_RESULTS.clear()

    in_maps = []
    for core in range(NCORES):
        m = {}
        for i in range(nslots):
            e, lo, hi = slot_lists[i][core]
            L = padded[i]
            n_s = hi - lo
            xg = np.zeros((H, L), dtype=ml_dtypes.bfloat16)
            cg = np.zeros((L,), dtype=np.float32)
            if n_s:
                xg[:, :n_s] = xT[:, token_sorted[lo:hi]]
                cg[:n_s] = gate_sorted[lo:hi]
            # pack k-tile pairs: [KH//2, 128, 2L], row = [k-even|k-odd]
            m[f"x{i}"] = np.ascontiguousarray(
                xg.reshape(KH // 2, 2, P, L).transpose(0, 2, 1, 3)
            ).reshape(KH // 2, P, 2 * L)
            m[f"c{i}"] = np.ascontiguousarray(np.broadcast_to(cg, (P, L)))
            m[f"w1_{i}"] = w1pk[e]
            m[f"w2_{i}"] = w2pk[e]
        in_maps.append(m)

    try:
        res = run_bass_kernel_spmd(nc, in_maps, list(range(NCORES)))
    except Exception:
        # transient device wedge has been observed to clear on retry
        time.sleep(2)
        res = run_bass_kernel_spmd(nc, in_maps, list(range(NCORES)))
    LAST_RESULTS.append(res)

    out = np.zeros((n_tok, hidden), dtype=np.float32)
    for core in range(NCORES):
        for i in range(nslots):
            e, lo, hi = slot_lists[i][core]
            n_s = hi - lo
            if n_s:
                y = res.results[core][f"y{i}"]       # [H, L] f32, scaled
                # tokens unique within one expert's list -> fancy add ok
                out[token_sorted[lo:hi]] += y[:, :n_s].T
    return out
